# revision 43
# baseline (speedup 1.0000x reference)
"""3-layer GCN (GCNConv x3) on 8 Trainium2 NeuronCores.

Strategy (dst-sharded, matmul aggregation):
  - GCN symmetric norm factorizes: norm_e = dinv[src]*dinv[dst], so
        out = dinv .* (A @ (dinv .* (x @ W))) + b
    with A the raw (unweighted, self-loop-augmented) adjacency.
  - Nodes sharded across 8 cores by contiguous row range (12500 each).
  - Per layer:
      1. GEMM  h~ = dinv .* (x @ W)  for the local shard  (TensorE)
      2. AllGather h~ into every core's HBM (collective)
      3. Edges (sorted by dst block of BLK nodes, sub-grouped by 32768-row
         src block so gather indices fit int16) are processed as 128-edge
         chunks: dma_gather pulls h~[src] rows into SBUF, the DVE builds a
         one-hot selection matrix M[e, j] = (dst_sel[e] == j) (int16
         compare -> DT), and TensorE accumulates
             psum[feat, node] += gathered^T @ M
         per dst block (two BLK-blocks share one [64,128] PSUM tile).
      4. Per-pair epilogue: multiply by dinv (DVE), +bias & ReLU (ScalarE),
         written feature-major so it feeds the next layer's GEMM directly.
  - Everything is statically unrolled and SPMD-uniform: per-(block, srcblk)
    edge counts are padded (gather idx 0 / dsel -1) to the max over cores.

Gather-pipeline tuning (the kernel is bound by SWDGE descriptor handling:
~3.4 ns/idx generation serial on GpSimd + ~35-65 ns/descriptor execution
serial per SDMA engine for random 256B HBM reads):
  - gathers round-robin over nq=4 SWDGE queues so each call's descriptor
    EXECUTION (drain) overlaps the next calls' GENERATION; a single queue
    serializes gen behind drain via ring await_space (14.0 -> 9.2 ms).
  - batch=1024 keeps every call at <=64 descriptors per SDMA engine, the
    limit for single_packet=True, which halves per-descriptor execution
    time. Larger batches (fewer, bigger calls) measured slower.
  - deep tile pools (nq+3 bufs) + sbn=5 superblocks keep 7 gather calls
    in flight. More bufs (9) intermittently hard-crashes the SDMA or
    silently corrupts a transfer (single-packet calls stacking on one
    queue ring) - do not raise.
  - gathers always fetch the FULL 256B padded row (elem_size ==
    elem_step == row_elems): the 128B-payload custom-emission path that
    16-bit rows otherwise take corrupts data intermittently (fp16 err
    2.4e-3..1.6e-2 run-to-run; full-row fp16 is bit-stable at 3.0e-4).
    With 256B payloads fp16 is NOT faster overall (7.1 vs 7.0 ms: the
    old "fp16 win" was the halved descriptor payload, i.e. the corrupt
    path), so fp32 ships. float32r at BLK=256 (1 cyc/row when
    >=256-wide) fails walrus codegen; fp32 at BLK=256 doubles matmul
    time.
  - epilogue stores issue from the Scalar engine: the SP sequencer issues
    DMAs in order, and epilogue stores waiting on a superblock's last
    matmul would head-of-line-block the meta-tile prefetches.

kernel(**inputs) is self-contained: host-side numpy planning, Bass build,
compile+run via run_bass_kernel_spmd on cores 0-7, gather + transpose out.
"""

import numpy as np

P = 128


def _cfg_full(dtype="float32"):
    return dict(
        n_nodes=100000,
        n_cores=8,
        d_in=128,
        d_hid=64,
        sblk=32768,  # src-block size for int16 gather indices (<= 32768)
        blk=128,  # dst-block size (one-hot width)
        sbn=5,  # dst blocks per superblock (sbn/2 live PSUM pair-tiles)
        no_pair_psum=True,  # BLK>=128 fills a psum tile; no pairing needed
        fp32r_agg=False,  # relaxed-fp32 agg matmuls (walrus codegen rejects)
        batch=1024,  # max gather-call size in indices (multiple of 128)
        dtype=dtype,  # "float32" or "bfloat16" for h~ / gather / matmul
        scratch=32768,  # SWDGE descriptor carveout bytes
        nq=4,  # SWDGE queues: gathers round-robin over cpu pairs
    )


def _np_dt(dtype):
    if dtype == "float32":
        return np.float32
    if dtype == "float16":
        return np.float16
    import ml_dtypes

    return ml_dtypes.bfloat16


# ----------------------------------------------------------------------------
# Host planning
# ----------------------------------------------------------------------------


def _host_plan(x, edge_index, cfg):
    """Numpy preprocessing: norm factorization, edge sorting/padding, the
    combined [gather-idx | dst-sel] int16 side-array in the SBUF layouts
    dma_gather expects, and the (SPMD-uniform) emission schedule."""
    N = cfg["n_nodes"]
    NCORES = cfg["n_cores"]
    SBLK = cfg["sblk"]
    BLK = cfg["blk"]
    SBN = cfg["sbn"]
    nloc = N // NCORES
    ntb = -(-nloc // BLK)  # dst blocks per core
    nsb = -(-ntb // SBN)  # superblocks per core
    nblk = -(-N // SBLK)  # src blocks
    ndt = _np_dt(cfg["dtype"])

    e0 = np.asarray(edge_index[0], dtype=np.int64)
    e1 = np.asarray(edge_index[1], dtype=np.int64)
    loop = np.arange(N, dtype=np.int64)
    src = np.concatenate([e0, loop])
    dst = np.concatenate([e1, loop])

    deg = np.bincount(dst, minlength=N).astype(np.float64)
    dinv = np.where(deg > 0, 1.0 / np.sqrt(deg), 0.0).astype(np.float32)

    core = dst // nloc
    dloc = dst - core * nloc
    dblock = dloc // BLK
    dsel = (dloc - dblock * BLK).astype(np.int16)
    s_of = src // SBLK
    gsrc = (src - s_of * SBLK).astype(np.int16)

    # counts per (core, dblock, srcblk); pad to max over cores, mult of 128
    key_cbs = (core * ntb + dblock) * nblk + s_of
    cnt = np.bincount(key_cbs, minlength=NCORES * ntb * nblk).reshape(
        NCORES, ntb, nblk
    )
    padded = ((cnt.max(axis=0) + P - 1) // P) * P  # [ntb, nblk]

    # stream order: (core, superblock, srcblk, dblock)
    sb_of = dblock // SBN
    skey = ((core * nsb + sb_of) * nblk + s_of) * ntb + dblock
    order = np.argsort(skey, kind="stable")
    g_sorted = gsrc[order]
    dsel_sorted = dsel[order]
    skey_cnt = np.bincount(skey, minlength=NCORES * nsb * nblk * ntb)
    skey_off = np.zeros(len(skey_cnt) + 1, dtype=np.int64)
    np.cumsum(skey_cnt, out=skey_off[1:])

    SL = int(padded.sum())  # uniform per-core stream length
    gidx_streams = np.zeros((NCORES, SL), np.int16)
    dsel_streams = np.full((NCORES, SL), -1, np.int16)

    pos_of = {}
    pos = 0
    for sb in range(nsb):
        blocks = range(sb * SBN, min((sb + 1) * SBN, ntb))
        for s in range(nblk):
            for b in blocks:
                pos_of[(b, s)] = pos
                pos += int(padded[b, s])
    assert pos == SL

    for c in range(NCORES):
        for sb in range(nsb):
            blocks = range(sb * SBN, min((sb + 1) * SBN, ntb))
            for s in range(nblk):
                for b in blocks:
                    k = ((c * nsb + sb) * nblk + s) * ntb + b
                    i0, i1 = int(skey_off[k]), int(skey_off[k + 1])
                    n = i1 - i0
                    q = pos_of[(b, s)]
                    gidx_streams[c, q : q + n] = g_sorted[i0:i1]
                    dsel_streams[c, q : q + n] = dsel_sorted[i0:i1]

    # Emission schedule (uniform across cores): per superblock a list of
    # gather calls (srcblk, n_idx, [dblock per chunk]).
    total_chunks = {b: int(padded[b].sum()) // P for b in range(ntb)}
    B = cfg["batch"]
    sched = []
    for sb in range(nsb):
        blocks = list(range(sb * SBN, min((sb + 1) * SBN, ntb)))
        per_s = []
        for s in range(nblk):
            chunk_blocks = []
            for b in blocks:
                chunk_blocks += [b] * (int(padded[b, s]) // P)
            qoff = pos_of[(blocks[0], s)]
            s_calls = []
            i = 0
            while i < len(chunk_blocks):
                take = min(B // P, len(chunk_blocks) - i)
                s_calls.append((s, take * P, chunk_blocks[i : i + take], qoff))
                qoff += take * P
                i += take
            per_s.append(s_calls)
        # interleave srcblk groups round-robin: consecutive calls (whose
        # drains overlap via the 4 SWDGE queues) then hit DIFFERENT 8MB
        # h_full regions, spreading DRAM pressure
        calls = []
        ptr = [0] * nblk
        while any(ptr[s] < len(per_s[s]) for s in range(nblk)):
            for s in range(nblk):
                if ptr[s] < len(per_s[s]):
                    calls.append(per_s[s][ptr[s]])
                    ptr[s] += 1
        sched.append((blocks, calls))

    # Combined meta array: per call [wrap16(gidx) | cols128(dsel)] int16.
    def wrap16(a):
        w = a.reshape(-1, 16).T
        return np.tile(w, (8, 1))

    def cols128(a):
        return a.reshape(-1, P).T

    metas = []
    for c in range(NCORES):
        parts = []
        q = 0
        for blocks, calls in sched:
            for s, n_idx, chunk_blocks, qs in calls:
                gi = gidx_streams[c, qs : qs + n_idx]
                ds = dsel_streams[c, qs : qs + n_idx]
                parts.append(wrap16(gi))
                parts.append(cols128(ds))
                q += n_idx
        assert q == SL
        metas.append(np.ascontiguousarray(np.concatenate(parts, axis=1)))
    TCM = metas[0].shape[1]

    per_core = []
    for c in range(NCORES):
        dv = np.zeros(ntb * BLK, np.float32)
        dv[:nloc] = dinv[c * nloc : (c + 1) * nloc]
        nt128 = -(-nloc // P)
        dvp = np.zeros(nt128 * P, np.float32)
        dvp[:nloc] = dinv[c * nloc : (c + 1) * nloc]
        dinv_cols = np.ascontiguousarray(dvp.reshape(nt128, P).T)  # [128, nt128]
        dinv_rep = np.tile(dvp.reshape(1, nt128 * P), (64, 1)).astype(np.float32)
        xt = np.ascontiguousarray(
            np.asarray(x[c * nloc : (c + 1) * nloc], dtype=np.float32).T
        ).astype(ndt)
        per_core.append(
            dict(
                xt=xt,
                dinv_cols=dinv_cols,
                dinv_rep=np.ascontiguousarray(dinv_rep),
                meta=metas[c],
            )
        )

    iota = np.tile(np.arange(BLK, dtype=np.int16), (P, 1))

    plan = dict(
        nloc=nloc,
        ntb=ntb,
        nsb=nsb,
        nblk=nblk,
        SL=SL,
        TCM=TCM,
        sched=sched,
        total_chunks=total_chunks,
        per_core=per_core,
        iota=iota,
        rows_s=[min(SBLK, N - s * SBLK) for s in range(nblk)],
    )
    return plan


# ----------------------------------------------------------------------------
# Device program
# ----------------------------------------------------------------------------


def _emit_gather(nc, out_ap, in_ap, idxs_ap, num_idxs, elem_size, elem_step, queue_num=0):
    """nc.gpsimd.dma_gather, or a direct emission when elem_size_bytes is not
    a multiple of 256 (that assert is a transpose-path restriction; the
    non-transpose HBM ucode only needs the row *stride* to be 256B-aligned)."""
    import concourse.mybir as mybir

    dt_size = mybir.dt.size(in_ap.dtype)
    if (elem_size * dt_size) % 256 == 0 and elem_step == elem_size:
        return nc.gpsimd.dma_gather(
            out_ap,
            in_ap,
            idxs_ap,
            num_idxs,
            num_idxs,
            elem_size,
            # single_packet coalesces each engine's whole descriptor stream
            # into one DMA packet; beyond ~64 descriptors that is out of spec
            # and hard-crashes the SDMA engine (NRT_EXEC_UNIT_UNRECOVERABLE).
            single_packet=num_idxs <= 1024,
            queue_num=queue_num,
        )
    g = nc.gpsimd
    stride_bytes = elem_step * dt_size
    assert stride_bytes % 256 == 0
    _in_ap = g.lower_ap_dma(in_ap, for_custom_bir_dma=True)
    _idxs_ap = g.lower_ap(idxs_ap)
    _out_ap = g.lower_ap(out_ap)
    return g.add_instruction(
        mybir.InstDMAGatherAnt(
            name=g.bass.get_next_instruction_name(),
            ins=[*_in_ap, _idxs_ap, g.lower_val_access(g.to_reg(num_idxs))],
            outs=[_out_ap],
            transpose=False,
            num_idxs=num_idxs,
            elem_size=elem_size,
            stride_bytes_256=stride_bytes // 256,
            gen_mode=0,
            single_packet=num_idxs <= 1024,
            queue_num=queue_num,
            sbuf_tokens_per_rank=0,
            sbuf_free_dim_per_rank=0,
            sbuf_free_dim_pad_per_rank=0,
            sbuf_byte_offset=0,
        )
    )


def _build_program(plan, cfg, b_nonzero, use_collective=True):
    import concourse.bacc as bacc
    import concourse.mybir as mybir
    import concourse.tile as tile

    dt = mybir.dt
    DT = {
        "float32": dt.float32,
        "float16": dt.float16,
        "bfloat16": dt.bfloat16,
    }[cfg["dtype"]]
    F32 = dt.float32
    DIN, DH = cfg["d_in"], cfg["d_hid"]
    N = cfg["n_nodes"]
    BLK = cfg["blk"]
    nloc, ntb, nblk = plan["nloc"], plan["ntb"], plan["nblk"]
    SL, TCM = plan["SL"], plan["TCM"]
    B = cfg["batch"]
    nt128 = -(-nloc // P)
    # padded row stride (elements of DT) for the gather source: byte stride
    # stays 256 so bf16 rows (128B payload) still gather legally.
    row_elems = 256 // dt.size(DT)

    nc = bacc.Bacc(
        None,
        target_bir_lowering=False,
        num_devices=cfg["n_cores"],
        dynamic_dma_scratch_size=cfg["scratch"],
        num_swdge_queues=cfg.get("nq", 1),
    )

    xt_in = nc.dram_tensor("xt", [DIN, nloc], DT, kind="ExternalInput")
    dinv_cols = nc.dram_tensor(
        "dinv_cols", [P, nt128], F32, kind="ExternalInput"
    )
    dinv_rep = nc.dram_tensor(
        "dinv_rep", [64, nt128 * P], F32, kind="ExternalInput"
    )
    ws_in = [
        nc.dram_tensor("w1", [DIN, DH], DT, kind="ExternalInput"),
        nc.dram_tensor("w2", [DH, DH], DT, kind="ExternalInput"),
        nc.dram_tensor("w3", [DH, DH], DT, kind="ExternalInput"),
    ]
    bs_in = nc.dram_tensor("bs", [64, 3], F32, kind="ExternalInput")
    meta_in = nc.dram_tensor("meta", [P, TCM], dt.int16, kind="ExternalInput")
    iota_in = nc.dram_tensor("iota", [P, BLK], dt.int16, kind="ExternalInput")
    out_dram = nc.dram_tensor("out", [DH, nloc], F32, kind="ExternalOutput")

    h_loc = nc.dram_tensor("h_loc", [nloc, row_elems], DT)
    h_full = nc.dram_tensor("h_full", [N, row_elems], DT, addr_space="Shared")
    xt2 = nc.dram_tensor("xt2", [DH, nloc], DT)
    xt3 = nc.dram_tensor("xt3", [DH, nloc], DT)

    rg = [list(range(cfg["n_cores"]))]
    MCOLS = B // 16 + B // P  # meta tile columns per call (max)

    if cfg.get("null_kernel"):
        # same I/O signature, ~no work: for calibrating dispatch overhead
        with tile.TileContext(nc) as tc:
            with tc.tile_pool(name="p", bufs=1) as pool:
                z = pool.tile([64, P], F32)
                nc.vector.memset(z[:, :], 0.0)
                nc.sync.dma_start(out=out_dram[:, :P], in_=z[:, :])
        nc.compile()
        return nc

    with tile.TileContext(nc) as tc:
        with (
            tc.tile_pool(name="const", bufs=1) as cpool,
            tc.tile_pool(name="work", bufs=3) as wpool,
            tc.tile_pool(name="gath", bufs=max(2, cfg.get("nq", 1) + 3)) as gpool,
            tc.tile_pool(name="onehot", bufs=max(2, cfg.get("nq", 1) + 3)) as mpool,
            tc.tile_pool(name="idx", bufs=max(2, cfg.get("nq", 1) + 3)) as ipool,
            tc.tile_pool(name="ps", bufs=2, space="PSUM") as pspool,
            tc.tile_pool(
                name="aggps",
                bufs=cfg["sbn"] // (1 if cfg.get("no_pair_psum") else 2) + 1,
                space="PSUM",
            ) as apool,
        ):
            w_sb = []
            for li, w in enumerate(ws_in):
                t = cpool.tile([w.shape[0], DH], DT, tag=f"w{li}")
                nc.sync.dma_start(out=t[:, :], in_=w[:, :])
                w_sb.append(t)
            b_sb = cpool.tile([64, 3], F32, tag="bs")
            nc.sync.dma_start(out=b_sb[:, :], in_=bs_in[:, :])
            dinvc_sb = cpool.tile([P, nt128], F32, tag="dinvc")
            nc.sync.dma_start(out=dinvc_sb[:, :], in_=dinv_cols[:, :])
            dinvr_sb = cpool.tile([64, nt128 * P], F32, tag="dinvr")
            nc.sync.dma_start(out=dinvr_sb[:, :], in_=dinv_rep[:, :])
            iota_sb = cpool.tile([P, BLK], dt.int16, tag="iota")
            nc.sync.dma_start(out=iota_sb[:, :], in_=iota_in[:, :])

            xt_srcs = [xt_in, xt2, xt3]
            xt_dsts = [xt2, xt3, None]

            for L in range(3):
                dk = DIN if L == 0 else DH
                xt_src = xt_srcs[L]
                # ---- phase 1: h~ = dinv .* (x @ W), two 128-row blocks/DMA
                for tp in range(-(-nt128 // 2)):
                    t0 = 2 * tp
                    nt_in_pair = min(2, nt128 - t0)
                    r0 = t0 * P
                    rows = min(2 * P, nloc - r0)
                    xts = wpool.tile([dk, 2 * P], DT, tag="xts")
                    nc.sync.dma_start(
                        out=xts[:, :rows], in_=xt_src[:, r0 : r0 + rows]
                    )
                    hs = wpool.tile([P, 2, DH], DT, tag="hs")
                    for j in range(nt_in_pair):
                        t = t0 + j
                        rt = min(P, nloc - t * P)
                        hp = pspool.tile([P, DH], F32, tag="hp")
                        nc.tensor.matmul(
                            hp[:rt, :],
                            lhsT=xts[:, j * P : j * P + rt],
                            rhs=w_sb[L][:, :],
                            start=True,
                            stop=True,
                        )
                        nc.scalar.activation(
                            hs[:rt, j, :],
                            hp[:rt, :],
                            mybir.ActivationFunctionType.Copy,
                            scale=dinvc_sb[:rt, t : t + 1],
                        )
                    if rows == 2 * P and not cfg.get("no_pair_store"):
                        # single DMA for both blocks via a strided dst AP
                        nc.sync.dma_start(
                            out=h_loc[r0 : r0 + rows, :DH].rearrange(
                                "(c p) f -> p c f", p=P
                            ),
                            in_=hs[:, :, :],
                        )
                    else:
                        for j in range(nt_in_pair):
                            t = t0 + j
                            rt = min(P, nloc - t * P)
                            nc.sync.dma_start(
                                out=h_loc[t * P : t * P + rt, :DH],
                                in_=hs[:rt, j, :],
                            )
                # ---- phase 2: AllGather
                if cfg.get("skip_coll"):
                    pass
                elif use_collective:
                    nc.gpsimd.collective_compute(
                        "AllGather",
                        mybir.AluOpType.bypass,
                        replica_groups=rg,
                        ins=[h_loc[:, :]],
                        outs=[h_full[:, :]],
                    )
                else:
                    nc.sync.dma_start(out=h_full[:nloc, :], in_=h_loc[:, :])
                # ---- phase 3: aggregation
                if cfg.get("skip_agg"):
                    continue
                mcol = 0
                ncall = 0
                seen = dict.fromkeys(range(ntb), 0)
                for blocks, calls in plan["sched"]:
                    pair_blocks = 1 if cfg.get("no_pair_psum") else 2
                    pair_tiles = {}
                    for b in blocks:
                        pr = b // pair_blocks
                        if pr not in pair_tiles:
                            pt = apool.tile(
                                [64, pair_blocks * BLK],
                                F32,
                                tag="aggps",
                                name=f"aggps{pr}",
                            )
                            if pair_blocks > 1:
                                # two independent half-column accumulation
                                # groups share the tile; a start=True reset
                                # would clobber the sibling half, so zero
                                # once and accumulate with start=False.
                                nc.vector.memset(pt[:, :], 0.0)
                            pair_tiles[pr] = pt
                    for s, n_idx, chunk_blocks, _qs in calls:
                        nch = n_idx // P
                        icols = n_idx // 16
                        mt = ipool.tile([P, MCOLS], dt.int16, tag="mt")
                        nc.sync.dma_start(
                            out=mt[:, : icols + nch],
                            in_=meta_in[:, mcol : mcol + icols + nch],
                        )
                        # gather the FULL 256B padded row (row_elems >= DH):
                        # elem_size == elem_step keeps 16-bit dtypes on the
                        # standard dma_gather path (the 128B-payload custom
                        # path showed run-to-run numeric instability)
                        gt = gpool.tile([P, B // P, row_elems], DT, tag="gt")
                        if cfg.get("skip_gather"):
                            pass
                        else:
                            _emit_gather(
                            nc,
                            gt[:, :nch, :],
                            h_full[
                                cfg["sblk"] * s : cfg["sblk"] * s
                                + plan["rows_s"][s],
                                :row_elems,
                            ],
                            mt[:, :icols],
                            n_idx,
                            row_elems,
                            row_elems,
                            queue_num=ncall % cfg.get("nq", 1),
                        )
                        ncall += 1
                        M = mpool.tile([P, B // P, BLK], DT, tag="M")
                        if cfg.get("skip_onehot"):
                            pass
                        else:
                            nc.vector.tensor_tensor(
                            out=M[:, :nch, :],
                            in0=iota_sb[:]
                            .rearrange("p (c f) -> p c f", c=1)
                            .to_broadcast([P, nch, BLK]),
                            in1=mt[:, icols : icols + nch]
                            .rearrange("p (c f) -> p c f", f=1)
                            .to_broadcast([P, nch, BLK]),
                            op=mybir.AluOpType.is_equal,
                        )
                        for ci, b in enumerate(chunk_blocks):
                            if cfg.get("skip_matmul"):
                                seen[b] += 1
                                continue
                            rt = min(BLK, nloc - b * BLK)
                            half = (b % pair_blocks) * BLK
                            # fp32r at >=256-wide output runs 1 cyc/row vs
                            # fp32's 4: keeps the agg matmuls from backing
                            # up behind the gather drains. M is 0/1 (exact
                            # in any format); only gt mantissas truncate.
                            r32 = (
                                cfg.get("fp32r_agg")
                                and DT == mybir.dt.float32
                                and rt >= 256
                            )
                            nc.tensor.matmul(
                                pair_tiles[b // pair_blocks][:, half : half + rt],
                                lhsT=gt[:, ci, :DH].bitcast(mybir.dt.float32r)
                                if r32
                                else gt[:, ci, :DH],
                                rhs=M[:, ci, :rt].bitcast(mybir.dt.float32r)
                                if r32
                                else M[:, ci, :rt],
                                start=(seen[b] == 0) if pair_blocks == 1 else False,
                                stop=(seen[b] == plan["total_chunks"][b] - 1),
                                skip_group_check=pair_blocks > 1,
                            )
                            seen[b] += 1
                        mcol += icols + nch
                    # ---- epilogue per pair tile, one BLK sub-chunk at a time
                    for pr, pt in pair_tiles.items():
                        for j in range(pair_blocks):
                            c0 = (pr * pair_blocks + j) * BLK
                            if c0 >= nloc:
                                break
                            rt = min(BLK, nloc - c0)
                            off = j * BLK
                            u = wpool.tile([64, BLK], F32, tag="u")
                            nc.vector.tensor_tensor(
                                out=u[:, :rt],
                                in0=pt[:, off : off + rt],
                                in1=dinvr_sb[:, c0 : c0 + rt],
                                op=mybir.AluOpType.mult,
                            )
                            if L < 2:
                                us = wpool.tile([64, BLK], DT, tag="us")
                                nc.scalar.activation(
                                    us[:, :rt],
                                    u[:, :rt],
                                    mybir.ActivationFunctionType.Relu,
                                    bias=b_sb[:, L : L + 1] if b_nonzero else 0.0,
                                )
                                nc.scalar.dma_start(
                                    out=xt_dsts[L][:, c0 : c0 + rt],
                                    in_=us[:, :rt],
                                )
                            else:
                                if b_nonzero:
                                    nc.vector.tensor_scalar(
                                        u[:, :rt],
                                        u[:, :rt],
                                        b_sb[:, L : L + 1],
                                        None,
                                        mybir.AluOpType.add,
                                    )
                                nc.scalar.dma_start(
                                    out=out_dram[:, c0 : c0 + rt], in_=u[:, :rt]
                                )
                assert mcol == TCM
    nc.compile()
    return nc


# ----------------------------------------------------------------------------
# Entry points
# ----------------------------------------------------------------------------


def build_and_run(inputs, cfg, trace=False):
    from concourse.bass_utils import run_bass_kernel_spmd

    x = np.asarray(inputs["x"])
    plan = _host_plan(x, np.asarray(inputs["edge_index"]), cfg)
    ndt = _np_dt(cfg["dtype"])

    bvals = [np.asarray(inputs[k], dtype=np.float32) for k in ("b1", "b2", "b3")]
    b_nonzero = any(np.any(b != 0) for b in bvals)
    bs = np.zeros((64, 3), np.float32)
    for i, b in enumerate(bvals):
        bs[: b.shape[0], i] = b

    nc = _build_program(plan, cfg, b_nonzero)

    ws = [
        np.ascontiguousarray(np.asarray(inputs[k], dtype=np.float32)).astype(ndt)
        for k in ("W1", "W2", "W3")
    ]
    in_maps = []
    for c in range(cfg["n_cores"]):
        pc = plan["per_core"][c]
        in_maps.append(
            {
                "xt": pc["xt"],
                "dinv_cols": pc["dinv_cols"],
                "dinv_rep": pc["dinv_rep"],
                "w1": ws[0],
                "w2": ws[1],
                "w3": ws[2],
                "bs": bs,
                "meta": pc["meta"],
                "iota": plan["iota"],
            }
        )

    res = run_bass_kernel_spmd(
        nc, in_maps, core_ids=list(range(cfg["n_cores"])), trace=trace
    )
    out = np.concatenate(
        [np.asarray(r["out"]).T for r in res.results], axis=0
    ).astype(np.float32)
    return out, res


def kernel(**inputs):
    # fp16 would be ~8% faster (4x matmul rate collapses the aggregation
    # backlog) but its max rel err is run-to-run unstable (2.4e-3 .. 1.6e-2
    # observed vs the 2e-2 gate) - ship bit-stable fp32 (4.96e-7).
    cfg = _cfg_full(dtype="float32")
    out, _ = build_and_run(inputs, cfg)
    return out



# revision 44
# speedup vs baseline: 1.0484x; 1.0484x over previous
"""3-layer GCN (GCNConv x3) on 8 Trainium2 NeuronCores.

Strategy (dst-sharded, matmul aggregation):
  - GCN symmetric norm factorizes: norm_e = dinv[src]*dinv[dst], so
        out = dinv .* (A @ (dinv .* (x @ W))) + b
    with A the raw (unweighted, self-loop-augmented) adjacency.
  - Nodes sharded across 8 cores by contiguous row range (12500 each).
  - Per layer:
      1. GEMM  h~ = dinv .* (x @ W)  for the local shard  (TensorE)
      2. AllGather h~ into every core's HBM (collective)
      3. Edges (sorted by dst block of BLK nodes, sub-grouped by 32768-row
         src block so gather indices fit int16) are processed as 128-edge
         chunks: dma_gather pulls h~[src] rows into SBUF, the DVE builds a
         one-hot selection matrix M[e, j] = (dst_sel[e] == j) (int16
         compare -> DT), and TensorE accumulates
             psum[feat, node] += gathered^T @ M
         per dst block (two BLK-blocks share one [64,128] PSUM tile).
      4. Per-pair epilogue: multiply by dinv (DVE), +bias & ReLU (ScalarE),
         written feature-major so it feeds the next layer's GEMM directly.
  - Everything is statically unrolled and SPMD-uniform: per-(block, srcblk)
    edge counts are padded (gather idx 0 / dsel -1) to the max over cores.

Gather-pipeline tuning (the kernel is bound by SWDGE descriptor handling:
~3.4 ns/idx generation serial on GpSimd + ~35-65 ns/descriptor execution
serial per SDMA engine for random 256B HBM reads):
  - gathers round-robin over nq=4 SWDGE queues so each call's descriptor
    EXECUTION (drain) overlaps the next calls' GENERATION; a single queue
    serializes gen behind drain via ring await_space (14.0 -> 9.2 ms).
  - batch=1024 keeps every call at <=64 descriptors per SDMA engine, the
    limit for single_packet=True, which halves per-descriptor execution
    time. Larger batches (fewer, bigger calls) measured slower.
  - deep tile pools (nq+3 bufs) + sbn=5 superblocks keep 7 gather calls
    in flight. More bufs (9) intermittently hard-crashes the SDMA or
    silently corrupts a transfer (single-packet calls stacking on one
    queue ring) - do not raise.
  - gathers always fetch the FULL 256B padded row (elem_size ==
    elem_step == row_elems): the 128B-payload custom-emission path that
    16-bit rows otherwise take corrupts data intermittently (fp16 err
    2.4e-3..1.6e-2 run-to-run; full-row fp16 is bit-stable at 3.0e-4).
    With 256B payloads fp16 is NOT faster overall (7.1 vs 7.0 ms: the
    old "fp16 win" was the halved descriptor payload, i.e. the corrupt
    path), so fp32 ships. float32r at BLK=256 (1 cyc/row when
    >=256-wide) fails walrus codegen; fp32 at BLK=256 doubles matmul
    time.
  - epilogue stores issue from the Scalar engine: the SP sequencer issues
    DMAs in order, and epilogue stores waiting on a superblock's last
    matmul would head-of-line-block the meta-tile prefetches.

kernel(**inputs) is self-contained: host-side numpy planning, Bass build,
compile+run via run_bass_kernel_spmd on cores 0-7, gather + transpose out.
"""

import numpy as np

P = 128


def _cfg_full(dtype="float32"):
    return dict(
        n_nodes=100000,
        n_cores=8,
        d_in=128,
        d_hid=64,
        sblk=32768,  # src-block size for int16 gather indices (<= 32768)
        blk=128,  # dst-block size (one-hot width)
        sbn=5,  # dst blocks per superblock (sbn/2 live PSUM pair-tiles)
        no_pair_psum=True,  # BLK>=128 fills a psum tile; no pairing needed
        fp32r_agg=False,  # relaxed-fp32 agg matmuls (walrus codegen rejects)
        batch=1024,  # max gather-call size in indices (multiple of 128)
        dtype=dtype,  # "float32" or "bfloat16" for h~ / gather / matmul
        scratch=32768,  # SWDGE descriptor carveout bytes
        nq=4,  # SWDGE queues: gathers round-robin over cpu pairs
    )


def _np_dt(dtype):
    if dtype == "float32":
        return np.float32
    if dtype == "float16":
        return np.float16
    import ml_dtypes

    return ml_dtypes.bfloat16


# ----------------------------------------------------------------------------
# Host planning
# ----------------------------------------------------------------------------


def _host_plan(x, edge_index, cfg):
    """Numpy preprocessing: norm factorization, edge sorting/padding, the
    combined [gather-idx | dst-sel] int16 side-array in the SBUF layouts
    dma_gather expects, and the (SPMD-uniform) emission schedule."""
    N = cfg["n_nodes"]
    NCORES = cfg["n_cores"]
    SBLK = cfg["sblk"]
    BLK = cfg["blk"]
    SBN = cfg["sbn"]
    nloc = N // NCORES
    ntb = -(-nloc // BLK)  # dst blocks per core
    nsb = -(-ntb // SBN)  # superblocks per core
    nblk = -(-N // SBLK)  # src blocks
    ndt = _np_dt(cfg["dtype"])

    e0 = np.asarray(edge_index[0], dtype=np.int64)
    e1 = np.asarray(edge_index[1], dtype=np.int64)
    loop = np.arange(N, dtype=np.int64)
    src = np.concatenate([e0, loop])
    dst = np.concatenate([e1, loop])

    deg = np.bincount(dst, minlength=N).astype(np.float64)
    dinv = np.where(deg > 0, 1.0 / np.sqrt(deg), 0.0).astype(np.float32)

    core = dst // nloc
    dloc = dst - core * nloc
    dblock = dloc // BLK
    dsel = (dloc - dblock * BLK).astype(np.int16)
    s_of = src // SBLK
    gsrc = (src - s_of * SBLK).astype(np.int16)

    # counts per (core, dblock, srcblk); pad to max over cores, mult of 128
    key_cbs = (core * ntb + dblock) * nblk + s_of
    cnt = np.bincount(key_cbs, minlength=NCORES * ntb * nblk).reshape(
        NCORES, ntb, nblk
    )
    padded = ((cnt.max(axis=0) + P - 1) // P) * P  # [ntb, nblk]

    # stream order: (core, superblock, srcblk, dblock)
    sb_of = dblock // SBN
    skey = ((core * nsb + sb_of) * nblk + s_of) * ntb + dblock
    order = np.argsort(skey, kind="stable")
    g_sorted = gsrc[order]
    dsel_sorted = dsel[order]
    skey_cnt = np.bincount(skey, minlength=NCORES * nsb * nblk * ntb)
    skey_off = np.zeros(len(skey_cnt) + 1, dtype=np.int64)
    np.cumsum(skey_cnt, out=skey_off[1:])

    SL = int(padded.sum())  # uniform per-core stream length
    gidx_streams = np.zeros((NCORES, SL), np.int16)
    dsel_streams = np.full((NCORES, SL), -1, np.int16)

    pos_of = {}
    pos = 0
    for sb in range(nsb):
        blocks = range(sb * SBN, min((sb + 1) * SBN, ntb))
        for s in range(nblk):
            for b in blocks:
                pos_of[(b, s)] = pos
                pos += int(padded[b, s])
    assert pos == SL

    for c in range(NCORES):
        for sb in range(nsb):
            blocks = range(sb * SBN, min((sb + 1) * SBN, ntb))
            for s in range(nblk):
                for b in blocks:
                    k = ((c * nsb + sb) * nblk + s) * ntb + b
                    i0, i1 = int(skey_off[k]), int(skey_off[k + 1])
                    n = i1 - i0
                    q = pos_of[(b, s)]
                    gidx_streams[c, q : q + n] = g_sorted[i0:i1]
                    dsel_streams[c, q : q + n] = dsel_sorted[i0:i1]

    # Emission schedule (uniform across cores): per superblock a list of
    # gather calls (srcblk, n_idx, [dblock per chunk]).
    total_chunks = {b: int(padded[b].sum()) // P for b in range(ntb)}
    B = cfg["batch"]
    sched = []
    for sb in range(nsb):
        blocks = list(range(sb * SBN, min((sb + 1) * SBN, ntb)))
        calls = []
        for s in range(nblk):
            chunk_blocks = []
            for b in blocks:
                chunk_blocks += [b] * (int(padded[b, s]) // P)
            i = 0
            while i < len(chunk_blocks):
                take = min(B // P, len(chunk_blocks) - i)
                calls.append((s, take * P, chunk_blocks[i : i + take]))
                i += take
        sched.append((blocks, calls))

    # Combined meta array: per call [wrap16(gidx) | cols128(dsel)] int16.
    def wrap16(a):
        w = a.reshape(-1, 16).T
        return np.tile(w, (8, 1))

    def cols128(a):
        return a.reshape(-1, P).T

    metas = []
    for c in range(NCORES):
        parts = []
        q = 0
        for blocks, calls in sched:
            for s, n_idx, chunk_blocks in calls:
                gi = gidx_streams[c, q : q + n_idx]
                ds = dsel_streams[c, q : q + n_idx]
                parts.append(wrap16(gi))
                parts.append(cols128(ds))
                q += n_idx
        assert q == SL
        metas.append(np.ascontiguousarray(np.concatenate(parts, axis=1)))
    TCM = metas[0].shape[1]

    per_core = []
    for c in range(NCORES):
        dv = np.zeros(ntb * BLK, np.float32)
        dv[:nloc] = dinv[c * nloc : (c + 1) * nloc]
        nt128 = -(-nloc // P)
        dvp = np.zeros(nt128 * P, np.float32)
        dvp[:nloc] = dinv[c * nloc : (c + 1) * nloc]
        dinv_cols = np.ascontiguousarray(dvp.reshape(nt128, P).T)  # [128, nt128]
        dinv_rep = np.tile(dvp.reshape(1, nt128 * P), (64, 1)).astype(np.float32)
        xt = np.ascontiguousarray(
            np.asarray(x[c * nloc : (c + 1) * nloc], dtype=np.float32).T
        ).astype(ndt)
        per_core.append(
            dict(
                xt=xt,
                dinv_cols=dinv_cols,
                dinv_rep=np.ascontiguousarray(dinv_rep),
                meta=metas[c],
            )
        )

    iota = np.tile(np.arange(BLK, dtype=np.int16), (P, 1))

    plan = dict(
        nloc=nloc,
        ntb=ntb,
        nsb=nsb,
        nblk=nblk,
        SL=SL,
        TCM=TCM,
        sched=sched,
        total_chunks=total_chunks,
        per_core=per_core,
        iota=iota,
        rows_s=[min(SBLK, N - s * SBLK) for s in range(nblk)],
    )
    return plan


# ----------------------------------------------------------------------------
# Device program
# ----------------------------------------------------------------------------


def _emit_gather(nc, out_ap, in_ap, idxs_ap, num_idxs, elem_size, elem_step, queue_num=0):
    """nc.gpsimd.dma_gather, or a direct emission when elem_size_bytes is not
    a multiple of 256 (that assert is a transpose-path restriction; the
    non-transpose HBM ucode only needs the row *stride* to be 256B-aligned)."""
    import concourse.mybir as mybir

    dt_size = mybir.dt.size(in_ap.dtype)
    if (elem_size * dt_size) % 256 == 0 and elem_step == elem_size:
        return nc.gpsimd.dma_gather(
            out_ap,
            in_ap,
            idxs_ap,
            num_idxs,
            num_idxs,
            elem_size,
            # single_packet coalesces each engine's whole descriptor stream
            # into one DMA packet; beyond ~64 descriptors that is out of spec
            # and hard-crashes the SDMA engine (NRT_EXEC_UNIT_UNRECOVERABLE).
            single_packet=num_idxs <= 1024,
            queue_num=queue_num,
        )
    g = nc.gpsimd
    stride_bytes = elem_step * dt_size
    assert stride_bytes % 256 == 0
    _in_ap = g.lower_ap_dma(in_ap, for_custom_bir_dma=True)
    _idxs_ap = g.lower_ap(idxs_ap)
    _out_ap = g.lower_ap(out_ap)
    return g.add_instruction(
        mybir.InstDMAGatherAnt(
            name=g.bass.get_next_instruction_name(),
            ins=[*_in_ap, _idxs_ap, g.lower_val_access(g.to_reg(num_idxs))],
            outs=[_out_ap],
            transpose=False,
            num_idxs=num_idxs,
            elem_size=elem_size,
            stride_bytes_256=stride_bytes // 256,
            gen_mode=0,
            single_packet=num_idxs <= 1024,
            queue_num=queue_num,
            sbuf_tokens_per_rank=0,
            sbuf_free_dim_per_rank=0,
            sbuf_free_dim_pad_per_rank=0,
            sbuf_byte_offset=0,
        )
    )


def _build_program(plan, cfg, b_nonzero, use_collective=True):
    import concourse.bacc as bacc
    import concourse.mybir as mybir
    import concourse.tile as tile

    dt = mybir.dt
    DT = {
        "float32": dt.float32,
        "float16": dt.float16,
        "bfloat16": dt.bfloat16,
    }[cfg["dtype"]]
    F32 = dt.float32
    DIN, DH = cfg["d_in"], cfg["d_hid"]
    N = cfg["n_nodes"]
    BLK = cfg["blk"]
    nloc, ntb, nblk = plan["nloc"], plan["ntb"], plan["nblk"]
    SL, TCM = plan["SL"], plan["TCM"]
    B = cfg["batch"]
    nt128 = -(-nloc // P)
    # padded row stride (elements of DT) for the gather source: byte stride
    # stays 256 so bf16 rows (128B payload) still gather legally.
    row_elems = 256 // dt.size(DT)

    nc = bacc.Bacc(
        None,
        target_bir_lowering=False,
        num_devices=cfg["n_cores"],
        dynamic_dma_scratch_size=cfg["scratch"],
        num_swdge_queues=cfg.get("nq", 1),
    )

    xt_in = nc.dram_tensor("xt", [DIN, nloc], DT, kind="ExternalInput")
    dinv_cols = nc.dram_tensor(
        "dinv_cols", [P, nt128], F32, kind="ExternalInput"
    )
    dinv_rep = nc.dram_tensor(
        "dinv_rep", [64, nt128 * P], F32, kind="ExternalInput"
    )
    ws_in = [
        nc.dram_tensor("w1", [DIN, DH], DT, kind="ExternalInput"),
        nc.dram_tensor("w2", [DH, DH], DT, kind="ExternalInput"),
        nc.dram_tensor("w3", [DH, DH], DT, kind="ExternalInput"),
    ]
    bs_in = nc.dram_tensor("bs", [64, 3], F32, kind="ExternalInput")
    meta_in = nc.dram_tensor("meta", [P, TCM], dt.int16, kind="ExternalInput")
    iota_in = nc.dram_tensor("iota", [P, BLK], dt.int16, kind="ExternalInput")
    out_dram = nc.dram_tensor("out", [DH, nloc], F32, kind="ExternalOutput")

    h_loc = nc.dram_tensor("h_loc", [nloc, row_elems], DT)
    h_full = nc.dram_tensor("h_full", [N, row_elems], DT, addr_space="Shared")
    xt2 = nc.dram_tensor("xt2", [DH, nloc], DT)
    xt3 = nc.dram_tensor("xt3", [DH, nloc], DT)

    rg = [list(range(cfg["n_cores"]))]
    MCOLS = B // 16 + B // P  # meta tile columns per call (max)

    if cfg.get("null_kernel"):
        # same I/O signature, ~no work: for calibrating dispatch overhead
        with tile.TileContext(nc) as tc:
            with tc.tile_pool(name="p", bufs=1) as pool:
                z = pool.tile([64, P], F32)
                nc.vector.memset(z[:, :], 0.0)
                nc.sync.dma_start(out=out_dram[:, :P], in_=z[:, :])
        nc.compile()
        return nc

    with tile.TileContext(nc) as tc:
        with (
            tc.tile_pool(name="const", bufs=1) as cpool,
            tc.tile_pool(name="work", bufs=3) as wpool,
            tc.tile_pool(name="gath", bufs=max(2, cfg.get("nq", 1) + 3)) as gpool,
            tc.tile_pool(name="onehot", bufs=max(2, cfg.get("nq", 1) + 3)) as mpool,
            tc.tile_pool(name="idx", bufs=max(2, cfg.get("nq", 1) + 3)) as ipool,
            tc.tile_pool(name="ps", bufs=2, space="PSUM") as pspool,
            tc.tile_pool(
                name="aggps",
                bufs=cfg["sbn"] // (1 if cfg.get("no_pair_psum") else 2) + 1,
                space="PSUM",
            ) as apool,
        ):
            w_sb = []
            for li, w in enumerate(ws_in):
                t = cpool.tile([w.shape[0], DH], DT, tag=f"w{li}")
                nc.sync.dma_start(out=t[:, :], in_=w[:, :])
                w_sb.append(t)
            b_sb = cpool.tile([64, 3], F32, tag="bs")
            nc.sync.dma_start(out=b_sb[:, :], in_=bs_in[:, :])
            dinvc_sb = cpool.tile([P, nt128], F32, tag="dinvc")
            nc.sync.dma_start(out=dinvc_sb[:, :], in_=dinv_cols[:, :])
            dinvr_sb = cpool.tile([64, nt128 * P], F32, tag="dinvr")
            nc.sync.dma_start(out=dinvr_sb[:, :], in_=dinv_rep[:, :])
            iota_sb = cpool.tile([P, BLK], dt.int16, tag="iota")
            nc.sync.dma_start(out=iota_sb[:, :], in_=iota_in[:, :])

            xt_srcs = [xt_in, xt2, xt3]
            xt_dsts = [xt2, xt3, None]

            for L in range(3):
                dk = DIN if L == 0 else DH
                xt_src = xt_srcs[L]
                # ---- phase 1: h~ = dinv .* (x @ W), two 128-row blocks/DMA
                for tp in range(-(-nt128 // 2)):
                    t0 = 2 * tp
                    nt_in_pair = min(2, nt128 - t0)
                    r0 = t0 * P
                    rows = min(2 * P, nloc - r0)
                    xts = wpool.tile([dk, 2 * P], DT, tag="xts")
                    nc.sync.dma_start(
                        out=xts[:, :rows], in_=xt_src[:, r0 : r0 + rows]
                    )
                    hs = wpool.tile([P, 2, DH], DT, tag="hs")
                    for j in range(nt_in_pair):
                        t = t0 + j
                        rt = min(P, nloc - t * P)
                        hp = pspool.tile([P, DH], F32, tag="hp")
                        nc.tensor.matmul(
                            hp[:rt, :],
                            lhsT=xts[:, j * P : j * P + rt],
                            rhs=w_sb[L][:, :],
                            start=True,
                            stop=True,
                        )
                        nc.scalar.activation(
                            hs[:rt, j, :],
                            hp[:rt, :],
                            mybir.ActivationFunctionType.Copy,
                            scale=dinvc_sb[:rt, t : t + 1],
                        )
                    if rows == 2 * P and not cfg.get("no_pair_store"):
                        # single DMA for both blocks via a strided dst AP
                        nc.sync.dma_start(
                            out=h_loc[r0 : r0 + rows, :DH].rearrange(
                                "(c p) f -> p c f", p=P
                            ),
                            in_=hs[:, :, :],
                        )
                    else:
                        for j in range(nt_in_pair):
                            t = t0 + j
                            rt = min(P, nloc - t * P)
                            nc.sync.dma_start(
                                out=h_loc[t * P : t * P + rt, :DH],
                                in_=hs[:rt, j, :],
                            )
                # ---- phase 2: AllGather
                if cfg.get("skip_coll"):
                    pass
                elif use_collective:
                    nc.gpsimd.collective_compute(
                        "AllGather",
                        mybir.AluOpType.bypass,
                        replica_groups=rg,
                        ins=[h_loc[:, :]],
                        outs=[h_full[:, :]],
                    )
                else:
                    nc.sync.dma_start(out=h_full[:nloc, :], in_=h_loc[:, :])
                # ---- phase 3: aggregation
                if cfg.get("skip_agg"):
                    continue
                mcol = 0
                ncall = 0
                seen = dict.fromkeys(range(ntb), 0)
                for blocks, calls in plan["sched"]:
                    pair_blocks = 1 if cfg.get("no_pair_psum") else 2
                    pair_tiles = {}
                    for b in blocks:
                        pr = b // pair_blocks
                        if pr not in pair_tiles:
                            pt = apool.tile(
                                [64, pair_blocks * BLK],
                                F32,
                                tag="aggps",
                                name=f"aggps{pr}",
                            )
                            if pair_blocks > 1:
                                # two independent half-column accumulation
                                # groups share the tile; a start=True reset
                                # would clobber the sibling half, so zero
                                # once and accumulate with start=False.
                                nc.vector.memset(pt[:, :], 0.0)
                            pair_tiles[pr] = pt
                    for s, n_idx, chunk_blocks in calls:
                        nch = n_idx // P
                        icols = n_idx // 16
                        mt = ipool.tile([P, MCOLS], dt.int16, tag="mt")
                        nc.sync.dma_start(
                            out=mt[:, : icols + nch],
                            in_=meta_in[:, mcol : mcol + icols + nch],
                        )
                        # gather the FULL 256B padded row (row_elems >= DH):
                        # elem_size == elem_step keeps 16-bit dtypes on the
                        # standard dma_gather path (the 128B-payload custom
                        # path showed run-to-run numeric instability)
                        gt = gpool.tile([P, B // P, row_elems], DT, tag="gt")
                        if cfg.get("skip_gather"):
                            pass
                        else:
                            _emit_gather(
                            nc,
                            gt[:, :nch, :],
                            h_full[
                                cfg["sblk"] * s : cfg["sblk"] * s
                                + plan["rows_s"][s],
                                :row_elems,
                            ],
                            mt[:, :icols],
                            n_idx,
                            row_elems,
                            row_elems,
                            queue_num=ncall % cfg.get("nq", 1),
                        )
                        ncall += 1
                        M = mpool.tile([P, B // P, BLK], DT, tag="M")
                        if cfg.get("skip_onehot"):
                            pass
                        else:
                            nc.vector.tensor_tensor(
                            out=M[:, :nch, :],
                            in0=iota_sb[:]
                            .rearrange("p (c f) -> p c f", c=1)
                            .to_broadcast([P, nch, BLK]),
                            in1=mt[:, icols : icols + nch]
                            .rearrange("p (c f) -> p c f", f=1)
                            .to_broadcast([P, nch, BLK]),
                            op=mybir.AluOpType.is_equal,
                        )
                        for ci, b in enumerate(chunk_blocks):
                            if cfg.get("skip_matmul"):
                                seen[b] += 1
                                continue
                            rt = min(BLK, nloc - b * BLK)
                            half = (b % pair_blocks) * BLK
                            # fp32r at >=256-wide output runs 1 cyc/row vs
                            # fp32's 4: keeps the agg matmuls from backing
                            # up behind the gather drains. M is 0/1 (exact
                            # in any format); only gt mantissas truncate.
                            r32 = (
                                cfg.get("fp32r_agg")
                                and DT == mybir.dt.float32
                                and rt >= 256
                            )
                            nc.tensor.matmul(
                                pair_tiles[b // pair_blocks][:, half : half + rt],
                                lhsT=gt[:, ci, :DH].bitcast(mybir.dt.float32r)
                                if r32
                                else gt[:, ci, :DH],
                                rhs=M[:, ci, :rt].bitcast(mybir.dt.float32r)
                                if r32
                                else M[:, ci, :rt],
                                start=(seen[b] == 0) if pair_blocks == 1 else False,
                                stop=(seen[b] == plan["total_chunks"][b] - 1),
                                skip_group_check=pair_blocks > 1,
                            )
                            seen[b] += 1
                        mcol += icols + nch
                    # ---- epilogue per pair tile, one BLK sub-chunk at a time
                    for pr, pt in pair_tiles.items():
                        for j in range(pair_blocks):
                            c0 = (pr * pair_blocks + j) * BLK
                            if c0 >= nloc:
                                break
                            rt = min(BLK, nloc - c0)
                            off = j * BLK
                            u = wpool.tile([64, BLK], F32, tag="u")
                            nc.vector.tensor_tensor(
                                out=u[:, :rt],
                                in0=pt[:, off : off + rt],
                                in1=dinvr_sb[:, c0 : c0 + rt],
                                op=mybir.AluOpType.mult,
                            )
                            if L < 2:
                                us = wpool.tile([64, BLK], DT, tag="us")
                                nc.scalar.activation(
                                    us[:, :rt],
                                    u[:, :rt],
                                    mybir.ActivationFunctionType.Relu,
                                    bias=b_sb[:, L : L + 1] if b_nonzero else 0.0,
                                )
                                nc.scalar.dma_start(
                                    out=xt_dsts[L][:, c0 : c0 + rt],
                                    in_=us[:, :rt],
                                )
                            else:
                                if b_nonzero:
                                    nc.vector.tensor_scalar(
                                        u[:, :rt],
                                        u[:, :rt],
                                        b_sb[:, L : L + 1],
                                        None,
                                        mybir.AluOpType.add,
                                    )
                                nc.scalar.dma_start(
                                    out=out_dram[:, c0 : c0 + rt], in_=u[:, :rt]
                                )
                assert mcol == TCM
    nc.compile()
    return nc


# ----------------------------------------------------------------------------
# Entry points
# ----------------------------------------------------------------------------


def build_and_run(inputs, cfg, trace=False):
    from concourse.bass_utils import run_bass_kernel_spmd

    x = np.asarray(inputs["x"])
    plan = _host_plan(x, np.asarray(inputs["edge_index"]), cfg)
    ndt = _np_dt(cfg["dtype"])

    bvals = [np.asarray(inputs[k], dtype=np.float32) for k in ("b1", "b2", "b3")]
    b_nonzero = any(np.any(b != 0) for b in bvals)
    bs = np.zeros((64, 3), np.float32)
    for i, b in enumerate(bvals):
        bs[: b.shape[0], i] = b

    nc = _build_program(plan, cfg, b_nonzero)

    ws = [
        np.ascontiguousarray(np.asarray(inputs[k], dtype=np.float32)).astype(ndt)
        for k in ("W1", "W2", "W3")
    ]
    in_maps = []
    for c in range(cfg["n_cores"]):
        pc = plan["per_core"][c]
        in_maps.append(
            {
                "xt": pc["xt"],
                "dinv_cols": pc["dinv_cols"],
                "dinv_rep": pc["dinv_rep"],
                "w1": ws[0],
                "w2": ws[1],
                "w3": ws[2],
                "bs": bs,
                "meta": pc["meta"],
                "iota": plan["iota"],
            }
        )

    res = run_bass_kernel_spmd(
        nc, in_maps, core_ids=list(range(cfg["n_cores"])), trace=trace
    )
    out = np.concatenate(
        [np.asarray(r["out"]).T for r in res.results], axis=0
    ).astype(np.float32)
    return out, res


def kernel(**inputs):
    # fp16 would be ~8% faster (4x matmul rate collapses the aggregation
    # backlog) but its max rel err is run-to-run unstable (2.4e-3 .. 1.6e-2
    # observed vs the 2e-2 gate) - ship bit-stable fp32 (4.96e-7).
    cfg = _cfg_full(dtype="float32")
    out, _ = build_and_run(inputs, cfg)
    return out

